# revision 30
# baseline (speedup 1.0000x reference)
"""Biased axial (tied) attention kernel for 8 Trainium2 NeuronCores.

Sharding: the score einsum contracts over the first L axis (n) of the
LN'd/transposed pair tensor.  Each core takes 48 of the 384 n-rows and
computes partial scores A[h,i,j] for ALL (i,j).  A per-i-chunk bf16
ReduceScatter (interleaved row ownership: core c owns rows
ic*128 + c*16 + t) gives each core full scores for 48 i-rows, which it
softmaxes locally (the bias projection for those rows is computed
locally from a host-reordered bias slice), and an AllGather of the bf16
attention weights redistributes the full attn tensor for the output
einsum.  Each core computes output columns k in its own n-shard, so the
output rows of the final (transposed) result are shard-contiguous.

LayerNorm is folded into the projections: mean subtraction is exact via
W2 = W' - colsum(W')/128 applied host-side to every projection weight,
so on-chip LN reduces to rs = rsqrt(var+eps) per position (pool_avg
stats, squares on the otherwise-idle gpsimd engine) and one broadcast
multiply rs*x.  Inputs are pre-cast to bf16 host-side to halve HBM
traffic.  All matmuls run in bf16 with fp32 PSUM accumulation.
"""

import functools
import math
from contextlib import ExitStack

import numpy as np
import ml_dtypes

import concourse.bacc as bacc
import concourse.mybir as mybir
from concourse.bass_utils import run_bass_kernel_spmd
from concourse.tile import TileContext

N_CORES = 8
L = 384
D = 128
H = 4
DH = 32
NL = L // N_CORES          # 48 rows per core
NCHUNK = L // 128          # 3
NI = 16                    # i-rows owned per core per 128-chunk (128/8)
NPOS = L * NL              # 18432 positions per LN'd tensor slice
EPS = 1e-5

F32 = mybir.dt.float32
BF16 = mybir.dt.bfloat16
AX = mybir.AxisListType
ALU = mybir.AluOpType
ACTF = mybir.ActivationFunctionType

RG = [list(range(N_CORES))]


def _emit_ln(nc, pools, src, lnfm):
    """Scale `src` (DRAM bf16 [L, NL, D], position-major) by per-position
    rsqrt(var+eps) and write bf16 feature-major into SBUF tile `lnfm`
    [128, NPOS] (pos = cc*6144 + n*128 + p).  Mean subtraction is folded
    into the projection weights (W2), so no mean handling here.
    """
    s6p, mvp, xccp = pools
    for cc in range(NCHUNK):
        xcc = xccp.tile([128, 48, D], BF16, tag="xcc")
        nc.gpsimd.dma_start(out=xcc[:], in_=src[cc * 128:(cc + 1) * 128, :, :])
        sq = s6p.tile([128, 48, D], BF16, tag="sq")
        nc.scalar.square(out=sq[:], in_=xcc[:])
        tA = mvp.tile([128, 48], F32, tag="tA")
        tB = mvp.tile([128, 48], F32, tag="tB")
        rs = mvp.tile([128, 48], F32, tag="rs")
        # var = mean(x^2) - mean(x)^2
        nc.vector.tensor_reduce(out=tA[:], in_=xcc[:], axis=AX.X, op=ALU.add)
        nc.vector.tensor_reduce(out=tB[:], in_=sq[:], axis=AX.X, op=ALU.add)
        nc.vector.tensor_scalar_mul(tA[:], tA[:], 1.0 / 128.0)
        nc.vector.tensor_mul(out=tA[:], in0=tA[:], in1=tA[:])
        nc.vector.scalar_tensor_tensor(
            out=tA[:], in0=tB[:], scalar=1.0 / 128.0, in1=tA[:],
            op0=ALU.mult, op1=ALU.subtract)
        nc.vector.tensor_scalar_add(tA[:], tA[:], EPS)
        nc.scalar.sqrt(out=tB[:], in_=tA[:])
        nc.vector.reciprocal(out=rs[:], in_=tB[:])
        rs_b = rs[:].unsqueeze(2).broadcast_to([128, 48, D])
        nc.vector.tensor_tensor(out=xcc[:], in0=xcc[:], in1=rs_b, op=ALU.mult)
        nc.scalar.dma_start(
            out=lnfm[:, cc * 6144:(cc + 1) * 6144]
                .rearrange("p (n j) -> p n j", j=128),
            in_=xcc.rearrange("p n j -> p (n j)"), transpose=True)


DEBUG_DUMP = False


@functools.lru_cache(maxsize=4)
def build_program(has_bo: bool):
    nc = bacc.Bacc(num_devices=N_CORES)

    xr = nc.declare_dram_parameter("xr", [L, NL, D], BF16, isOutput=False)
    xc = nc.declare_dram_parameter("xc", [L, NL, D], BF16, isOutput=False)
    xb = nc.declare_dram_parameter("xb", [L, NL, D], BF16, isOutput=False)
    wq = nc.declare_dram_parameter("wq", [D, D], BF16, isOutput=False)
    wk = nc.declare_dram_parameter("wk", [D, D], BF16, isOutput=False)
    wv = nc.declare_dram_parameter("wv", [D, D], BF16, isOutput=False)
    wg = nc.declare_dram_parameter("wg", [D, D], BF16, isOutput=False)
    wo = nc.declare_dram_parameter("wo", [D, D], BF16, isOutput=False)
    wb = nc.declare_dram_parameter("wb", [D, H], BF16, isOutput=False)
    cq = nc.declare_dram_parameter("cq", [D, 1], F32, isOutput=False)
    ck = nc.declare_dram_parameter("ck", [D, 1], F32, isOutput=False)
    cv = nc.declare_dram_parameter("cv", [D, 1], F32, isOutput=False)
    cg = nc.declare_dram_parameter("cg", [D, 1], F32, isOutput=False)
    bo_b = nc.declare_dram_parameter("bo_b", [D, D], F32, isOutput=False)
    out = nc.declare_dram_parameter("out", [NL, L, D], F32, isOutput=True)

    if DEBUG_DUMP:
        dbg_apart = nc.declare_dram_parameter("dbg_apart", [L, H, L], BF16,
                                              isOutput=True)
        dbg_ars = nc.declare_dram_parameter("dbg_ars", [NL, H, L], BF16,
                                            isOutput=True)
        dbg_att = nc.declare_dram_parameter("dbg_att", [NL, H, L], BF16,
                                            isOutput=True)
        dbg_bp = nc.declare_dram_parameter("dbg_bp", [H, NL, L], F32,
                                           isOutput=True)
        dbg_attnT = nc.declare_dram_parameter("dbg_attnT", [128, H * NCHUNK, L],
                                              BF16, isOutput=True)
        dbg_vpm = nc.declare_dram_parameter("dbg_vpm", [128, NL * NCHUNK, D],
                                            BF16, isOutput=True)
        dbg_gate = nc.declare_dram_parameter("dbg_gate", [128, NPOS], BF16,
                                             isOutput=True)

    a_part = nc.dram_tensor("a_part", [L, H, L], BF16)
    a_rs = nc.dram_tensor("a_rs", [NL, H, L], BF16)
    bp_part = nc.dram_tensor("bp_part", [H, NL, L], F32)
    att_my = nc.dram_tensor("att_my", [NL, H, L], BF16)
    att_all = nc.dram_tensor("att_all", [N_CORES, NL, H, L], BF16,
                             addr_space="Shared")

    with TileContext(nc) as tc, ExitStack() as es:
        cpool = es.enter_context(tc.tile_pool(name="consts", bufs=1))
        wq_sb = cpool.tile([D, D], BF16, tag="wq")
        wk_sb = cpool.tile([D, D], BF16, tag="wk")
        wv_sb = cpool.tile([D, D], BF16, tag="wv")
        wg_sb = cpool.tile([D, D], BF16, tag="wg")
        wo_sb = cpool.tile([D, D], BF16, tag="wo")
        wb_sb = cpool.tile([D, H], BF16, tag="wb")
        cq_sb = cpool.tile([D, 1], F32, tag="cq")
        ck_sb = cpool.tile([D, 1], F32, tag="ck")
        cv_sb = cpool.tile([D, 1], F32, tag="cv")
        cg_sb = cpool.tile([D, 1], F32, tag="cg")
        for t, s in [(wq_sb, wq), (wk_sb, wk), (wv_sb, wv), (wg_sb, wg),
                     (wo_sb, wo), (wb_sb, wb), (cq_sb, cq), (ck_sb, ck),
                     (cv_sb, cv), (cg_sb, cg)]:
            nc.sync.dma_start(out=t[:], in_=s[:])
        if has_bo:
            bo_sb = cpool.tile([D, D], F32, tag="bo")
            nc.sync.dma_start(out=bo_sb[:], in_=bo_b[:])

        lnp = es.enter_context(tc.tile_pool(name="lnfm", bufs=1))
        bigp = es.enter_context(tc.tile_pool(name="big", bufs=1))
        es_ln = ExitStack()
        s6p = es_ln.enter_context(tc.tile_pool(name="s6", bufs=1))
        mvp = es_ln.enter_context(tc.tile_pool(name="mv", bufs=2))
        xccp = es_ln.enter_context(tc.tile_pool(name="xcc", bufs=2))
        ln_pools = (s6p, mvp, xccp)

        # ---- phase 1: LN of pair rows (n-shard) -> lnfm1
        lnfm1 = lnp.tile([128, NPOS], BF16, tag="lnfm")
        _emit_ln(nc, ln_pools, xr, lnfm1)

        # ---- phase 2: q/k/v projections (feature-major)
        q_sb = bigp.tile([128, NPOS], BF16, tag="bigq")
        k_sb = bigp.tile([128, NPOS], BF16, tag="bigk")
        v_pm = bigp.tile([128, NL * NCHUNK, D], BF16, tag="bigvpm")
        with tc.tile_pool(name="ppsum", bufs=3, space="PSUM") as ppsum, \
             tc.tile_pool(name="vrot", bufs=4) as vrotp:
            for dst, w_sb, c_sb, eng in ((q_sb, wq_sb, cq_sb, "act"),
                                         (k_sb, wk_sb, ck_sb, "dve")):
                for ch in range(NPOS // 512):
                    ps = ppsum.tile([128, 512], F32, tag="pps")
                    sl = slice(ch * 512, (ch + 1) * 512)
                    nc.tensor.matmul(ps[:], lhsT=w_sb[:], rhs=lnfm1[:, sl],
                                     start=True, stop=True)
                    if eng == "act":
                        nc.scalar.activation(dst[:, sl], ps[:], ACTF.Identity,
                                             bias=c_sb[:, 0:1])
                    else:
                        nc.vector.tensor_scalar_add(dst[:, sl], ps[:],
                                                    c_sb[:, 0:1])
            # v: groups of 2 n-rows; batched xbar transpose per group
            for n2 in range(NL // 2):
                ps2 = ppsum.tile([128, 2, 512], F32, tag="vps", bufs=2)
                lnv = lnfm1.rearrange("f (cc n j) -> f cc n j", cc=NCHUNK, n=48)
                for g in range(2):
                    n = n2 * 2 + g
                    nc.tensor.matmul(ps2[:, g, :L],
                                     lhsT=wv_sb[:],
                                     rhs=lnv[:, :, n, :],
                                     start=True, stop=True)
                vr2 = vrotp.tile([128, 2, L], BF16, tag="vrot")
                nc.scalar.activation(vr2[:], ps2[:, :, :L], ACTF.Identity,
                                     bias=cv_sb[:, 0:1])
                nc.sync.dma_start(
                    out=v_pm[:, n2 * 6:(n2 + 1) * 6, :],
                    in_=vr2.rearrange("p n j -> p (n j)"), transpose=True)

        # All later phases reuse phase-2's PSUM banks and SBUF scratch;
        # cross-queue WAR tracking is not airtight on HW, so fence here.
        tc.strict_bb_all_engine_barrier()

        # ---- phase 3: gate LN (overlaps scores)
        lnfm3 = lnp.tile([128, NPOS], BF16, tag="lnfm")
        _emit_ln(nc, ln_pools, xc, lnfm3)

        # ---- phase 4: scores A[i, j] per head, K=32 row-tiled over n;
        #      per-chunk bf16 ReduceScatter starts while later chunks compute
        with tc.tile_pool(name="apsum", bufs=2, space="PSUM") as apsum, \
             tc.tile_pool(name="asb", bufs=2) as asbp:
            for ic in range(NCHUNK):
                aps = [apsum.tile([128, L], F32, tag=f"A{h}", name=f"A{h}") for h in range(H)]
                for n in range(NL):
                    kv = k_sb.rearrange("f (cc n j) -> f cc n j", cc=NCHUNK, n=48)
                    for h in range(H):
                        nc.tensor.matmul(
                            aps[h][:],
                            lhsT=q_sb[32 * h:32 * (h + 1),
                                      ic * 6144 + n * 128: ic * 6144 + (n + 1) * 128],
                            rhs=kv[32 * h:32 * (h + 1), :, n, :],
                            start=(n == 0), stop=(n == NL - 1),
                            tile_position=(32 * h, 0))
                a_sb = asbp.tile([128, H, L], BF16, tag="asb")
                for h in range(H):
                    if h % 2 == 0:
                        nc.vector.tensor_copy(a_sb[:, h, :], aps[h][:])
                    else:
                        nc.scalar.copy(a_sb[:, h, :], aps[h][:])
                nc.sync.dma_start(
                    out=a_part[ic * 128:(ic + 1) * 128], in_=a_sb[:])
                nc.gpsimd.collective_compute(
                    "ReduceScatter", ALU.add, replica_groups=RG,
                    ins=[a_part[ic * 128:(ic + 1) * 128]],
                    outs=[a_rs[ic * NI:(ic + 1) * NI]])

        # ---- phase 5: gate projection (overlaps the ReduceScatter)
        gate_sb = bigp.tile([128, NPOS], BF16, tag="bigq")
        with tc.tile_pool(name="gpsum", bufs=4, space="PSUM") as gpsum:
            for ch in range(NPOS // 512):
                ps = gpsum.tile([128, 512], F32, tag="gps")
                sl = slice(ch * 512, (ch + 1) * 512)
                nc.tensor.matmul(ps[:], lhsT=wg_sb[:], rhs=lnfm3[:, sl],
                                 start=True, stop=True)
                nc.scalar.activation(gate_sb[:, sl], ps[:], ACTF.Sigmoid,
                                     bias=cg_sb[:, 0:1])

        # Fence: phase-6 PSUM reuses the gate projection's banks.
        tc.strict_bb_all_engine_barrier()

        # ---- phase 6: bias LN + bproj (overlaps the ReduceScatter)
        lnfm2 = lnp.tile([128, NPOS], BF16, tag="lnfm")
        _emit_ln(nc, ln_pools, xb, lnfm2)
        with tc.tile_pool(name="bpps", bufs=4, space="PSUM") as bpps, \
             tc.tile_pool(name="bpsbp", bufs=2) as bpsbp:
            for cc in range(NCHUNK):
                for w in range(3):
                    bp_t = bpsbp.tile([H, 2048], F32, tag="bpt")
                    for c4 in range(4):
                        ps = bpps.tile([H, 512], F32, tag="bpps")
                        sl = slice(cc * 6144 + w * 2048 + c4 * 512,
                                   cc * 6144 + w * 2048 + (c4 + 1) * 512)
                        nc.tensor.matmul(ps[:], lhsT=wb_sb[:], rhs=lnfm2[:, sl],
                                         start=True, stop=True)
                        if c4 % 2 == 0:
                            nc.vector.tensor_copy(
                                bp_t[:, c4 * 512:(c4 + 1) * 512], ps[:])
                        else:
                            nc.scalar.copy(
                                bp_t[:, c4 * 512:(c4 + 1) * 512], ps[:])
                    nc.sync.dma_start(
                        out=bp_part[:, w * 16:(w + 1) * 16, cc * 128:(cc + 1) * 128],
                        in_=bp_t.rearrange("h (il j) -> h il j", j=128))

        es_ln.close()
        # Fence: softmax tiles reuse the LN scratch space.
        tc.strict_bb_all_engine_barrier()

        # ---- phase 7: local softmax over the core's 48 owned i-rows
        with tc.tile_pool(name="smp", bufs=1) as smp, \
             tc.tile_pool(name="sms", bufs=1) as sms:
            bp_i = smp.tile([NL, H, L], F32, tag="bpi")
            nc.sync.dma_start(out=bp_i[:],
                              in_=bp_part.rearrange("h i j -> i h j"))
            a48 = smp.tile([NL, H, L], BF16, tag="a48")
            nc.sync.dma_start(out=a48[:], in_=a_rs[:])
            af = smp.tile([NL, H, L], F32, tag="af")
            nc.vector.tensor_add(out=af[:], in0=a48[:], in1=bp_i[:])
            nm = sms.tile([NL, H], F32, tag="nm")
            nc.vector.tensor_reduce(out=nm[:], in_=af[:], axis=AX.X,
                                    op=ALU.max)
            nmn = sms.tile([NL, H], F32, tag="nmn")
            nc.vector.tensor_scalar_mul(nmn[:], nm[:], -1.0)
            e48 = smp.tile([NL, H, L], F32, tag="e48")
            ssum = sms.tile([NL, H], F32, tag="ssum")
            for h in range(H):
                nc.scalar.activation(e48[:, h, :], af[:, h, :], ACTF.Exp,
                                     bias=nmn[:, h:h + 1],
                                     accum_out=ssum[:, h:h + 1])
            rsum = sms.tile([NL, H], F32, tag="rsum")
            nc.vector.reciprocal(out=rsum[:], in_=ssum[:])
            att48 = smp.tile([NL, H, L], BF16, tag="att48")
            rs_b = rsum[:].unsqueeze(2).broadcast_to([NL, H, L])
            nc.vector.tensor_tensor(out=att48[:], in0=e48[:], in1=rs_b,
                                    op=ALU.mult)
            nc.sync.dma_start(out=att_my[:], in_=att48[:])

        # ---- phase 8: AllGather of bf16 attention weights
        nc.gpsimd.collective_compute(
            "AllGather", ALU.bypass, replica_groups=RG,
            ins=[att_my[:]], outs=[att_all[:]])

        # ---- phase 9: attn transpose to [j, h, i] layout
        attnT = bigp.tile([128, H * NCHUNK, L], BF16, tag="bigattnT")
        with tc.tile_pool(name="atp", bufs=2) as atp:
            for ic in range(NCHUNK):
                i0 = ic * 128
                at_ch = atp.tile([128, H, L], BF16, tag="atch")
                for cpr in range(N_CORES):
                    nc.sync.dma_start(
                        out=at_ch[cpr * NI:(cpr + 1) * NI, :, :],
                        in_=att_all[cpr, ic * NI:(ic + 1) * NI])
                nc.sync.dma_start(
                    out=attnT[:, :, i0:i0 + 128],
                    in_=at_ch.rearrange("p h j -> p (h j)"), transpose=True)

        # ---- phase 10: output einsum (col-tiled by head) + gate + out proj
        with tc.tile_pool(name="opsum", bufs=3, space="PSUM") as opsum, \
             tc.tile_pool(name="fpsum", bufs=3, space="PSUM") as fpsum, \
             tc.tile_pool(name="ogp", bufs=3) as ogp, \
             tc.tile_pool(name="fsbp", bufs=2) as fsbp:
            for k in range(NL):
                ops_t = opsum.tile([128, L], F32, tag="ops")
                for jc in range(NCHUNK):
                    for h in range(H):
                        nc.tensor.matmul(
                            ops_t[32 * h:32 * (h + 1), :],
                            lhsT=v_pm[:, k * NCHUNK + jc, 32 * h:32 * (h + 1)],
                            rhs=attnT[:, h * NCHUNK + jc, :],
                            start=(jc == 0), stop=(jc == NCHUNK - 1),
                            tile_position=(0, 32 * h),
                            skip_group_check=True)
                og = ogp.tile([128, L], BF16, tag="og")
                gv = gate_sb.rearrange("f (cc n j) -> f cc n j", cc=NCHUNK, n=48)
                nc.vector.scalar_tensor_tensor(
                    out=og.rearrange("f (cc j) -> f cc j", cc=NCHUNK),
                    in0=ops_t.rearrange("f (cc j) -> f cc j", cc=NCHUNK),
                    scalar=1.0, in1=gv[:, :, k, :],
                    op0=ALU.mult, op1=ALU.mult)
                fps = fpsum.tile([128, NCHUNK, D], F32, tag="fps")
                for pc in range(NCHUNK):
                    nc.tensor.matmul(fps[:, pc, :],
                                     lhsT=og[:, pc * 128:(pc + 1) * 128],
                                     rhs=wo_sb[:], start=True, stop=True)
                if k % 4 == 0:
                    fsb4 = fsbp.tile([128, 4, NCHUNK, D], F32, tag="fsb")
                kk = k % 4
                if has_bo:
                    for pc in range(NCHUNK):
                        nc.vector.tensor_add(out=fsb4[:, kk, pc, :],
                                             in0=fps[:, pc, :], in1=bo_sb[:])
                elif k % 2 == 0:
                    nc.vector.tensor_copy(fsb4[:, kk, :, :], fps[:])
                else:
                    nc.scalar.copy(fsb4[:, kk, :, :], fps[:])
                if kk == 3:
                    k0 = k - 3
                    nc.sync.dma_start(
                        out=out[k0:k0 + 4]
                            .rearrange("k (pc p) d -> p (k pc) d", p=128),
                        in_=fsb4.rearrange("p k pc d -> p (k pc) d"))

        if DEBUG_DUMP:
            nc.sync.dma_start(out=dbg_apart[:], in_=a_part[:])
            nc.sync.dma_start(out=dbg_ars[:], in_=a_rs[:])
            nc.sync.dma_start(out=dbg_att[:], in_=att_my[:])
            nc.sync.dma_start(out=dbg_bp[:], in_=bp_part[:])
            nc.sync.dma_start(out=dbg_attnT[:], in_=attnT[:])
            nc.sync.dma_start(out=dbg_vpm[:], in_=v_pm[:])
            nc.sync.dma_start(out=dbg_gate[:], in_=gate_sb[:])

    nc.compile()
    return nc


def _fold(Wm, lnw, scale):
    Wp = (lnw[:, None] * Wm * scale).astype(np.float64)
    W2 = Wp - Wp.sum(axis=0, keepdims=True) / float(D)
    return W2.astype(ml_dtypes.bfloat16)


def _owned_rows(c):
    return [ic * 128 + c * NI + t for ic in range(NCHUNK) for t in range(NI)]


def _prep_inputs(pair, bias, ln_pair_w, ln_pair_b, ln_bias_w, ln_bias_b,
                 Wq, Wk, Wv, Wb, Wg, bg, Wo, bo):
    bf = ml_dtypes.bfloat16
    scaling = 1.0 / math.sqrt(DH)
    kscale = 1.0 / math.sqrt(L)
    wq_e = _fold(Wq, ln_pair_w, scaling)
    wk_e = _fold(Wk, ln_pair_w, kscale)
    wv_e = _fold(Wv, ln_pair_w, 1.0)
    wg_e = _fold(Wg, ln_pair_w, 1.0)
    wb_e = _fold(Wb, ln_bias_w, 1.0)
    wo_e = Wo.astype(bf)
    cq_e = (ln_pair_b @ (Wq * scaling)).astype(np.float32).reshape(D, 1)
    ck_e = (ln_pair_b @ (Wk * kscale)).astype(np.float32).reshape(D, 1)
    cv_e = (ln_pair_b @ Wv).astype(np.float32).reshape(D, 1)
    cg_e = (bg + ln_pair_b @ Wg).astype(np.float32).reshape(D, 1)
    bo_f = np.asarray(bo, np.float32)
    has_bo = bool(np.any(bo_f != 0.0))
    bo_bcast = np.broadcast_to(bo_f, (D, D)).copy() if has_bo \
        else np.zeros((D, D), np.float32)

    pair_bf = np.asarray(pair[0], bf)
    bias_bf = np.asarray(bias[0], bf)
    common = dict(wq=wq_e, wk=wk_e, wv=wv_e, wg=wg_e, wo=wo_e, wb=wb_e,
                  cq=cq_e, ck=ck_e, cv=cv_e, cg=cg_e, bo_b=bo_bcast)
    in_maps = []
    for c in range(N_CORES):
        r0 = c * NL
        m = dict(common)
        m["xr"] = np.ascontiguousarray(pair_bf[:, r0:r0 + NL, :])
        m["xc"] = np.ascontiguousarray(
            pair_bf[r0:r0 + NL, :, :].transpose(1, 0, 2))
        m["xb"] = np.ascontiguousarray(bias_bf[:, _owned_rows(c), :])
        in_maps.append(m)
    return in_maps, has_bo


TRACE = False
LAST_EXEC_NS = None
LAST_TRACE_DIR = None


def kernel(**inputs):
    global LAST_EXEC_NS, LAST_TRACE_DIR
    inputs = {k: np.asarray(v) for k, v in inputs.items()}
    in_maps, has_bo = _prep_inputs(**inputs)
    nc = build_program(has_bo)
    res = run_bass_kernel_spmd(nc, in_maps, list(range(N_CORES)), trace=TRACE)
    if TRACE:
        LAST_EXEC_NS = res.exec_time_ns
        if res.instructions_and_trace is not None:
            LAST_TRACE_DIR = res.instructions_and_trace[1]
    full = np.concatenate([res.results[c]["out"] for c in range(N_CORES)],
                          axis=0)[None]
    return full.astype(np.float32)


if __name__ == "__main__":
    nc = build_program(False)
    print("build ok")
